# revision 1
# baseline (speedup 1.0000x reference)
"""CondConv2d Trainium2 kernel.

B=32, C=192, H=W=64, O=192, E=8, 3x3 'same' conv.
Data-parallel over batch: 8 cores x 4 samples. Expert weights replicated.

Per-core pipeline:
  Phase A: stream x, per-(sample,channel) sums (global avg pool numerator).
  Routing: logits = sums @ routing_w.T; sigmoid(logits/4096 + b) on ACT.
  Phase B: mix expert weights with a block-diagonal PE matmul
           (K = 16 o-groups x 8 experts = 128 -> 16x fewer streamed columns
           than the naive K=8 mixing), partition-remap via SBUF->SBUF DMA,
           then PE transposes to per-sample lhsT[c, (kk,o)] layout.
  Phase C: conv = 9 shifted float32r matmuls accumulated in PSUM per
           8-row output strip; evict via DVE; DMA out.
"""

import sys
import numpy as np

for _p in ("/opt/trn_rl_repo",):
    if _p not in sys.path:
        sys.path.insert(0, _p)

BS = 4          # samples per core
C = 192
H = W = 64
O = 192
E = 8
KK = 9          # 3x3
CKK = C * KK    # 1728, per-o flattened (c,kh,kw) block in expert_weight rows
N_CORES = 8

# mixing pass structure: o-groups of 8 o-values
OSUB = 8
G1 = 16         # pass 1: o in [0,128)
G2 = 8          # pass 2: o in [128,192)
GLEN = OSUB * CKK          # 13824 elements of an expert row per group
MIXN = 432                 # mixing matmul free dim (>=256 keeps f32r at 1 cyc/col)
T_PER_G = GLEN // MIXN     # 32

_COMPILED = None


def _build():
    import concourse.bass as bass
    import concourse.bacc as bacc
    import concourse.mybir as mybir
    import concourse.tile as tile
    from concourse import masks

    f32 = mybir.dt.float32
    f32r = mybir.dt.float32r
    AX = mybir.AxisListType
    ACT = mybir.ActivationFunctionType

    nc = bacc.Bacc("TRN2", target_bir_lowering=False, debug=False)

    x_d = nc.dram_tensor("x", [BS, C, H, W], f32, kind="ExternalInput")
    x_r = nc.dram_tensor("x_r", [BS, C, H, W], f32r, kind="ExternalInput")
    ew_d = nc.dram_tensor("expert_weight", [E, O * CKK], f32r, kind="ExternalInput")
    rw_d = nc.dram_tensor("routing_w", [E, C], f32, kind="ExternalInput")
    rb_d = nc.dram_tensor("routing_b", [E], f32, kind="ExternalInput")
    zp_d = nc.dram_tensor("zpad", [128, 128], f32r, kind="ExternalInput")
    out_d = nc.dram_tensor("out", [BS, O, H, W], f32, kind="ExternalOutput")

    with tile.TileContext(nc) as tc:
        with (
            tc.tile_pool(name="persist", bufs=1) as persist,
            tc.tile_pool(name="psum_small", bufs=1, space="PSUM") as psum_small,
        ):
            # ---------- persistent small tiles ----------
            ident = persist.tile([128, 128], f32)
            masks.make_identity(nc, ident[:])

            rwT_lo = persist.tile([128, E], f32)
            rwT_hi = persist.tile([64, E], f32)
            rwT_src = rw_d.ap().rearrange("e c -> c e")
            nc.sync.dma_start(rwT_lo[:], rwT_src[0:128])
            nc.sync.dma_start(rwT_hi[:], rwT_src[128:192])
            rb_t = persist.tile([E, 1], f32)
            nc.sync.dma_start(rb_t[:], rb_d.ap().unsqueeze(1))

            pooled_lo = persist.tile([128, BS], f32)
            pooled_hi = persist.tile([64, BS], f32)

            # per-sample conv weights, transposed layout [c, kk*192+o]
            lhsT_lo = [persist.tile([128, KK * O], f32r, name=f"lhsT_lo{b}") for b in range(BS)]
            lhsT_hi = [persist.tile([64, KK * O], f32r, name=f"lhsT_hi{b}") for b in range(BS)]

            # ---------- Phase A: x sums for global avg pool ----------
            with tc.tile_pool(name="apool", bufs=2) as apool:
                for s in range(BS):
                    xa_lo = apool.tile([128, H, W], f32, tag="xa_lo")
                    nc.sync.dma_start(xa_lo[:], x_d[s, 0:128])
                    nc.vector.reduce_sum(pooled_lo[:, s : s + 1], xa_lo[:], axis=AX.XY)
                    xa_hi = apool.tile([64, H, W], f32, tag="xa_hi")
                    nc.sync.dma_start(xa_hi[:], x_d[s, 128:192])
                    nc.vector.reduce_sum(pooled_hi[:, s : s + 1], xa_hi[:], axis=AX.XY)

            # ---------- routing ----------
            psum_r = psum_small.tile([E, BS], f32)
            nc.tensor.matmul(psum_r[:], rwT_lo[:], pooled_lo[:], start=True, stop=False)
            nc.tensor.matmul(psum_r[:], rwT_hi[:], pooled_hi[:], start=False, stop=True)
            r_sb = persist.tile([E, BS], f32r)
            nc.scalar.activation(
                r_sb[:], psum_r[:], ACT.Sigmoid, bias=rb_t[:], scale=1.0 / (H * W)
            )

            # block-diagonal mixing weights bd[(g,e), (g,b)] = r[b,e]
            bd1 = persist.tile([G1 * E, G1 * BS], f32r)
            bd2 = persist.tile([G2 * E, G2 * BS], f32r)
            nc.sync.dma_start(bd1[:], zp_d.ap()[: G1 * E, : G1 * BS])
            nc.sync.dma_start(bd2[:], zp_d.ap()[: G2 * E, : G2 * BS])
            for g in range(G1):
                nc.sync.dma_start(
                    bd1[g * E : (g + 1) * E, g * BS : (g + 1) * BS], r_sb[:]
                )
            for g in range(G2):
                nc.sync.dma_start(
                    bd2[g * E : (g + 1) * E, g * BS : (g + 1) * BS], r_sb[:]
                )

            # ---------- Phase B: mix + remap + transpose ----------
            with (
                tc.tile_pool(name="spool", bufs=1) as spool,
                tc.tile_pool(name="natpool", bufs=1) as natpool,
                tc.tile_pool(name="epool", bufs=8) as epool,
                tc.tile_pool(name="mixpsum", bufs=3, space="PSUM") as mixpsum,
                tc.tile_pool(name="tpsum", bufs=3, space="PSUM") as tpsum,
            ):
                nat_lo = [natpool.tile([128, CKK], f32, name=f"nat_lo{b}") for b in range(BS)]
                nat_hi = [natpool.tile([64, CKK], f32, name=f"nat_hi{b}") for b in range(BS)]

                for pidx, (G, bd, nat, o_base) in enumerate(
                    ((G1, bd1, nat_lo, 0), (G2, bd2, nat_hi, 128))
                ):
                    S_all = spool.tile([G * BS, GLEN], f32, tag="S_all", name=f"S{pidx}")
                    ew_view = ew_d.ap()[:, o_base * CKK : (o_base + G * OSUB) * CKK]
                    ew_view = ew_view.rearrange("e (g u) -> g e u", g=G)
                    for t in range(T_PER_G):
                        ewt = epool.tile([G * E, MIXN], f32r, tag="ewt")
                        nc.sync.dma_start(ewt[:], ew_view[:, :, t * MIXN : (t + 1) * MIXN])
                        pm = mixpsum.tile([G * BS, MIXN], f32, tag="pm")
                        nc.tensor.matmul(
                            pm[:], bd[:], ewt[:],
                            start=True, stop=True,
                        )
                        nc.vector.tensor_copy(S_all[:, t * MIXN : (t + 1) * MIXN], pm[:])
                    # partition remap: row (g*BS+b) cols (o_sub,c,kk) -> nat[b][o, (c,kk)]
                    for b in range(BS):
                        for g in range(G):
                            nc.sync.dma_start(
                                nat[b][g * OSUB : (g + 1) * OSUB, :],
                                S_all[g * BS + b : g * BS + b + 1, :].rearrange(
                                    "p (o u) -> p o u", o=OSUB
                                ),
                            )

                # transposes: nat[b][o, (c,kk)] -> lhsT[b][c, (kk,o)]
                for b in range(BS):
                    for kk in range(KK):
                        for cc, (c0, c_n, lhsT) in enumerate(
                            ((0, 128, lhsT_lo[b]), (128, 64, lhsT_hi[b]))
                        ):
                            for oc, (o0, o_n, nat) in enumerate(
                                ((0, 128, nat_lo[b]), (128, 64, nat_hi[b]))
                            ):
                                src = nat[:].rearrange("o (c k) -> o c k", k=KK)[
                                    :, c0 : c0 + c_n, kk
                                ]
                                tp = tpsum.tile([128, 128], f32, tag="tp")
                                nc.tensor.transpose(
                                    tp[:c_n, :o_n], src, ident[:o_n, :o_n]
                                )
                                nc.scalar.copy(
                                    lhsT[:c_n, kk * O + o0 : kk * O + o0 + o_n],
                                    tp[:c_n, :o_n],
                                )

            # ---------- Phase C: conv ----------
            NS = 8          # row strips per sample
            SR = H // NS    # 8 output rows per strip
            with (
                tc.tile_pool(name="cpool", bufs=3) as cpool,
                tc.tile_pool(name="stgpool", bufs=3) as stgpool,
                tc.tile_pool(name="cpsum", bufs=3, space="PSUM") as cpsum,
            ):
                for s in range(BS):
                    for u in range(NS):
                        h0 = u * SR
                        strips = []
                        for cc, (c0, c_n) in enumerate(((0, 128), (128, 64))):
                            st = cpool.tile([c_n, SR + 2, W + 2], f32r, tag=f"strip{cc}")
                            nc.sync.dma_start(st[:, :, 0:1], zp_d.ap()[:c_n, 0 : SR + 2])
                            nc.sync.dma_start(st[:, :, W + 1 : W + 2], zp_d.ap()[:c_n, 0 : SR + 2])
                            if u == 0:
                                nc.sync.dma_start(st[:, 0:1, 1 : W + 1], zp_d.ap()[:c_n, 0:W])
                                nc.sync.dma_start(
                                    st[:, 1 : SR + 2, 1 : W + 1],
                                    x_r[s, c0 : c0 + c_n, 0 : SR + 1, :],
                                )
                            elif u == NS - 1:
                                nc.sync.dma_start(
                                    st[:, SR + 1 : SR + 2, 1 : W + 1], zp_d.ap()[:c_n, 0:W]
                                )
                                nc.sync.dma_start(
                                    st[:, 0 : SR + 1, 1 : W + 1],
                                    x_r[s, c0 : c0 + c_n, h0 - 1 : H, :],
                                )
                            else:
                                nc.sync.dma_start(
                                    st[:, :, 1 : W + 1],
                                    x_r[s, c0 : c0 + c_n, h0 - 1 : h0 + SR + 1, :],
                                )
                            strips.append((c0, c_n, st))

                        for oc, (o0, o_n) in enumerate(((0, 128), (128, 64))):
                            pc = cpsum.tile([o_n, SR, W], f32, tag=f"pc{oc}")
                            n_acc = KK * 2
                            i = 0
                            for kk in range(KK):
                                kh, kw = divmod(kk, 3)
                                for c0, c_n, st in strips:
                                    lt = lhsT_lo[s] if c0 == 0 else lhsT_hi[s]
                                    nc.tensor.matmul(
                                        pc[:],
                                        lt[:c_n, kk * O + o0 : kk * O + o0 + o_n],
                                        st[:c_n, kh : kh + SR, kw : kw + W],
                                        start=(i == 0),
                                        stop=(i == n_acc - 1),
                                    )
                                    i += 1
                            stg = stgpool.tile([o_n, SR, W], f32, tag=f"stg{oc}")
                            nc.vector.tensor_copy(stg[:], pc[:])
                            nc.sync.dma_start(
                                out_d[s, o0 : o0 + o_n, h0 : h0 + SR, :], stg[:]
                            )

    nc.compile()
    return nc


def _get_compiled():
    global _COMPILED
    if _COMPILED is None:
        _COMPILED = _build()
    return _COMPILED


def kernel(x, expert_weight, routing_w, routing_b, trace=False):
    from concourse.bass_utils import run_bass_kernel_spmd

    nc = _get_compiled()
    ew = np.ascontiguousarray(expert_weight, dtype=np.float32)
    _ZPAD = np.zeros((128, 128), dtype=np.float32)
    rw = np.ascontiguousarray(routing_w, dtype=np.float32)
    rb = np.ascontiguousarray(routing_b, dtype=np.float32)
    in_maps = [
        {
            "x": np.ascontiguousarray(x[i * BS : (i + 1) * BS], dtype=np.float32),
            "x_r": np.ascontiguousarray(x[i * BS : (i + 1) * BS], dtype=np.float32),
            "expert_weight": ew,
            "zpad": _ZPAD,
            "routing_w": rw,
            "routing_b": rb,
        }
        for i in range(N_CORES)
    ]
    res = run_bass_kernel_spmd(
        nc, in_maps, core_ids=list(range(N_CORES)), trace=trace
    )
    out = np.concatenate([res.results[i]["out"] for i in range(N_CORES)], axis=0)
    if trace:
        kernel.last_results = res
    return out



# revision 6
# speedup vs baseline: 1.5302x; 1.5302x over previous
"""CondConv2d Trainium2 kernel (v2).

B=32, C=192, H=W=64, O=192, E=8, 3x3 'same' conv.
Data-parallel over batch: 8 cores x 4 samples. Expert weights replicated.

Key design points vs v1:
  - x is zero-padded to 66x66 on the host (bf16) and stays SBUF-resident:
    one DMA per (sample, channel-half) serves BOTH the avg-pool reduction
    and all conv strip views.  No per-strip DMAs, no pad DMAs.
  - Channels 128..191 are host-packed twice into one 128-partition tile
    (row-shift 0 in partitions 0..63, row-shift +1 in partitions 64..127),
    so taps (kh=0,kh=1) of the hi channel group run as ONE full-contract
    matmul: 15 conv matmuls per strip/o-group instead of 18.
  - expert_weight is host-permuted into mixing order [(g,e) rows,
    (c_sub,taps,o) cols] so the mixing matmul + one partition-split DMA per
    sample directly yields the conv lhsT layout.  No PE transposes.
  - The block-diagonal mixing matrix bd[(g,e),(g,b)] = r[b,e] is built with
    one tiny matmul (identity-tiled lhsT, broadcast rhs) + one masked DVE
    multiply instead of 24 small DMAs.
  - Everything 16-bit on the wire (bf16); PSUM accumulation is f32.
"""

import sys
import numpy as np

for _p in ("/opt/trn_rl_repo",):
    if _p not in sys.path:
        sys.path.insert(0, _p)

import ml_dtypes

BF16 = ml_dtypes.bfloat16

BS = 4          # samples per core
C = 192
H = W = 64
HP = WP = 66    # padded
O = 192
E = 8
KK = 9          # 3x3
N_CORES = 8

G = 16          # mixing channel groups
CSUB = C // G // 1  # 12? no: lo covers 128 ch with CSUB=8 -> see below
CSUB = 8        # channels per group (both passes use 16 groups of 8 partitions)
LO_SPAN = KK * O            # 1728 per-partition cols of lhsT_lo
HI_SPAN = 6 * O             # 1152 per-partition cols of lhsT_hi
LO_COLS = CSUB * LO_SPAN    # 13824 streamed cols, lo mixing pass
HI_COLS = CSUB * HI_SPAN    # 9216 streamed cols, hi mixing pass
MIXN = 512                  # mixing matmul free dim (27 lo tiles, 18 hi tiles)

NS = 8          # row strips per sample
SR = H // NS    # 8 output rows per strip
QROWS = 32      # output rows staged per DMA (4 strips)

_COMPILED = None


def _build():
    import concourse.bass as bass
    import concourse.bacc as bacc
    import concourse.mybir as mybir
    import concourse.tile as tile

    f32 = mybir.dt.float32
    bf16 = mybir.dt.bfloat16
    AX = mybir.AxisListType
    ACT = mybir.ActivationFunctionType
    MUL = mybir.AluOpType.mult

    nc = bacc.Bacc("TRN2", target_bir_lowering=False, debug=False)

    xlo_d = nc.dram_tensor("xlo", [BS, 128, HP, WP], bf16, kind="ExternalInput")
    xhi_d = nc.dram_tensor("xhi", [BS, 128, HP, WP], bf16, kind="ExternalInput")
    ewlo_d = nc.dram_tensor("ewlo", [128, LO_COLS], bf16, kind="ExternalInput")
    ewhi_d = nc.dram_tensor("ewhi", [128, HI_COLS], bf16, kind="ExternalInput")
    rwT_d = nc.dram_tensor("rwT", [C, E], f32, kind="ExternalInput")
    rb_d = nc.dram_tensor("rb", [E], f32, kind="ExternalInput")
    irep_d = nc.dram_tensor("irep", [E, 128], bf16, kind="ExternalInput")
    mask_d = nc.dram_tensor("maskbd", [128, G * BS], bf16, kind="ExternalInput")
    out_d = nc.dram_tensor("out", [BS, O, H, W], f32, kind="ExternalOutput")

    with tile.TileContext(nc) as tc:
        with tc.tile_pool(name="persist", bufs=1) as persist:
            # ---------- constants ----------
            rwT_lo = persist.tile([128, E], f32)
            rwT_hi = persist.tile([64, E], f32)
            nc.sync.dma_start(rwT_lo[:], rwT_d.ap()[0:128])
            nc.sync.dma_start(rwT_hi[:], rwT_d.ap()[128:192])
            rb_t = persist.tile([E, 1], f32)
            nc.sync.dma_start(rb_t[:], rb_d.ap().unsqueeze(1))
            irep = persist.tile([E, 128], bf16)
            nc.sync.dma_start(irep[:], irep_d.ap())
            maskbd = persist.tile([128, G * BS], bf16)
            nc.sync.dma_start(maskbd[:], mask_d.ap())

            pooled_lo = persist.tile([128, BS], f32)
            pooled_hi = persist.tile([64, BS], f32)

            # ---------- resident x (+ Phase A reductions) ----------
            xs_lo = [persist.tile([128, HP, WP], bf16, name=f"xs_lo{b}") for b in range(BS)]
            xs_hi = [persist.tile([128, HP, WP], bf16, name=f"xs_hi{b}") for b in range(BS)]
            for b in range(BS):
                nc.sync.dma_start(xs_lo[b][:], xlo_d[b])
                nc.sync.dma_start(xs_hi[b][:], xhi_d[b])

            # ---------- resident expert weights (mixing layout) ----------
            ew_lo_sb = persist.tile([128, LO_COLS], bf16)
            ew_hi_sb = persist.tile([128, HI_COLS], bf16)
            nc.sync.dma_start(ew_lo_sb[:], ewlo_d.ap())
            nc.sync.dma_start(ew_hi_sb[:], ewhi_d.ap())

            for b in range(BS):
                nc.vector.reduce_sum(pooled_lo[:, b : b + 1], xs_lo[b][:], axis=AX.XY)
                nc.vector.reduce_sum(pooled_hi[:, b : b + 1], xs_hi[b][0:64], axis=AX.XY)

            # per-sample conv weights, already transposed layout
            lhsT_lo = [persist.tile([128, LO_SPAN], bf16, name=f"lhsT_lo{b}") for b in range(BS)]
            lhsT_hi = [persist.tile([128, HI_SPAN], bf16, name=f"lhsT_hi{b}") for b in range(BS)]

            with (
                tc.tile_pool(name="spool", bufs=1) as spool,
                tc.tile_pool(name="mixpsum", bufs=3, space="PSUM") as mixpsum,
                tc.tile_pool(name="smallpsum", bufs=1, space="PSUM") as smallpsum,
            ):
                # ---------- routing ----------
                psum_r = smallpsum.tile([E, BS], f32, tag="psum_r")
                nc.tensor.matmul(psum_r[:], rwT_lo[:], pooled_lo[:], start=True, stop=False)
                nc.tensor.matmul(psum_r[:], rwT_hi[:], pooled_hi[:], start=False, stop=True)
                r_sb = persist.tile([E, BS], bf16)
                nc.scalar.activation(
                    r_sb[:], psum_r[:], ACT.Sigmoid, bias=rb_t[:], scale=1.0 / (H * W)
                )

                # ---------- block-diagonal bd[(g,e), (g,b)] = r[b, e] ----------
                psum_bd = smallpsum.tile([128, G * BS], f32, tag="psum_bd")
                r_bcast = r_sb[:].unsqueeze(1).broadcast_to([E, G, BS])
                nc.tensor.matmul(psum_bd[:], irep[:], r_bcast, start=True, stop=True)
                bd = persist.tile([128, G * BS], bf16)
                nc.vector.tensor_tensor(bd[:], psum_bd[:], maskbd[:], MUL)

                # ---------- mixing ----------
                S_lo = spool.tile([G * BS, LO_COLS], bf16, name="S_lo")
                S_hi = spool.tile([G * BS, HI_COLS], bf16, name="S_hi")
                for t in range(LO_COLS // MIXN):
                    pm = mixpsum.tile([G * BS, MIXN], f32, tag="pm")
                    nc.tensor.matmul(
                        pm[:], bd[:], ew_lo_sb[:, t * MIXN : (t + 1) * MIXN],
                        start=True, stop=True,
                    )
                    nc.vector.tensor_copy(S_lo[:, t * MIXN : (t + 1) * MIXN], pm[:])
                # remap rows (g,b) -> lhsT_lo[b] partitions (g, c_sub)
                for b in range(BS):
                    src = S_lo[:].rearrange(
                        "(g b) (c u) -> g b c u", b=BS, c=CSUB
                    )[:, b]
                    nc.sync.dma_start(lhsT_lo[b][:], src)

                for t in range(HI_COLS // MIXN):
                    pm = mixpsum.tile([G * BS, MIXN], f32, tag="pm")
                    nc.tensor.matmul(
                        pm[:], bd[:], ew_hi_sb[:, t * MIXN : (t + 1) * MIXN],
                        start=True, stop=True,
                    )
                    nc.vector.tensor_copy(S_hi[:, t * MIXN : (t + 1) * MIXN], pm[:])
                for b in range(BS):
                    src = S_hi[:].rearrange(
                        "(g b) (c u) -> g b c u", b=BS, c=CSUB
                    )[:, b]
                    nc.sync.dma_start(lhsT_hi[b][:], src)

            # ---------- conv ----------
            with (
                tc.tile_pool(name="stgpool", bufs=2) as stgpool,
                tc.tile_pool(name="cpsum", bufs=3, space="PSUM") as cpsum,
            ):
                for s in range(BS):
                    for q in range(NS // 4):     # staged output groups of 4 strips
                        stgs = {}
                        for oc, (o0, o_n) in enumerate(((0, 128), (128, 64))):
                            stgs[oc] = stgpool.tile(
                                [o_n, QROWS, W], f32, tag=f"stg{oc}", name=f"stg{oc}"
                            )
                        for ui in range(4):
                            u = q * 4 + ui
                            h0 = u * SR
                            for oc, (o0, o_n) in enumerate(((0, 128), (128, 64))):
                                pc = cpsum.tile([o_n, SR, W], f32, tag=f"pc{oc}")
                                i = 0
                                n_acc = 15
                                for kh in range(3):
                                    for kw in range(3):
                                        kk = 3 * kh + kw
                                        nc.tensor.matmul(
                                            pc[:],
                                            lhsT_lo[s][:, kk * O + o0 : kk * O + o0 + o_n],
                                            xs_lo[s][:, h0 + kh : h0 + kh + SR, kw : kw + W],
                                            start=(i == 0), stop=(i == n_acc - 1),
                                        )
                                        i += 1
                                for a_idx, a in enumerate((0, 2)):
                                    for kw in range(3):
                                        j = 3 * a_idx + kw
                                        nc.tensor.matmul(
                                            pc[:],
                                            lhsT_hi[s][:, j * O + o0 : j * O + o0 + o_n],
                                            xs_hi[s][:, h0 + a : h0 + a + SR, kw : kw + W],
                                            start=(i == 0), stop=(i == n_acc - 1),
                                        )
                                        i += 1
                                nc.vector.tensor_copy(
                                    stgs[oc][:, ui * SR : (ui + 1) * SR, :], pc[:]
                                )
                        for oc, (o0, o_n) in enumerate(((0, 128), (128, 64))):
                            nc.sync.dma_start(
                                out_d[s, o0 : o0 + o_n, q * QROWS : (q + 1) * QROWS, :],
                                stgs[oc][:],
                            )

    nc.compile()
    return nc


def _get_compiled():
    global _COMPILED
    if _COMPILED is None:
        _COMPILED = _build()
    return _COMPILED


def _host_prep(x, expert_weight, routing_w, routing_b):
    """Pure layout transforms (pad/transpose/cast); no model arithmetic."""
    B = x.shape[0]
    x = np.asarray(x, dtype=np.float32)

    # padded lo channels [B, 128, 66, 66]
    xlo = np.zeros((B, 128, HP, WP), dtype=np.float32)
    xlo[:, :, 1 : H + 1, 1 : W + 1] = x[:, 0:128]
    # hi channels padded, plus row-shifted copy stacked on partitions 64..127
    xhi_p = np.zeros((B, 64, HP, WP), dtype=np.float32)
    xhi_p[:, :, 1 : H + 1, 1 : W + 1] = x[:, 128:192]
    xhi_r = np.zeros_like(xhi_p)
    xhi_r[:, :, 0 : HP - 1, :] = xhi_p[:, :, 1:HP, :]
    xhi = np.concatenate([xhi_p, xhi_r], axis=1)

    xlo = xlo.astype(BF16)
    xhi = xhi.astype(BF16)

    # expert weights -> [E, C, KH, KW, O]
    Wt = np.ascontiguousarray(
        np.asarray(expert_weight, dtype=np.float32)
        .reshape(E, O, C, 3, 3)
        .transpose(0, 2, 3, 4, 1)
    )
    # lo: channels 0..127, per-channel (kh, kw, o); mixing-partition-major:
    # SBUF partition (g*8+e) holds expert e's channels g*8..g*8+7
    ewlo = (
        np.ascontiguousarray(Wt[:, 0:128])
        .reshape(E, G, CSUB * LO_SPAN)
        .transpose(1, 0, 2)
        .reshape(128, LO_COLS)
        .astype(BF16)
    )
    # hi: bank0 = (kh0 taps, kh2 taps) for c128..191; bank1 = (kh1 taps, zeros)
    hi = Wt[:, 128:192]                       # [E, 64, 3, 3, O]
    bank0 = np.concatenate([hi[:, :, 0], hi[:, :, 2]], axis=2)   # [E, 64, 6, O]
    bank1 = np.concatenate(
        [hi[:, :, 1], np.zeros((E, 64, 3, O), dtype=np.float32)], axis=2
    )
    ewhi = (
        np.concatenate([bank0, bank1], axis=1)  # [E, 128, 6, O]
        .reshape(E, G, CSUB * HI_SPAN)
        .transpose(1, 0, 2)
        .reshape(128, HI_COLS)
        .astype(BF16)
    )

    rwT = np.ascontiguousarray(np.asarray(routing_w, dtype=np.float32).T)
    rb = np.ascontiguousarray(np.asarray(routing_b, dtype=np.float32))

    irep = np.ascontiguousarray(np.tile(np.eye(E, dtype=np.float32), (1, G))).astype(BF16)
    maskbd = np.zeros((128, G * BS), dtype=np.float32)
    for g in range(G):
        maskbd[g * E : (g + 1) * E, g * BS : (g + 1) * BS] = 1.0
    maskbd = maskbd.astype(BF16)

    return xlo, xhi, ewlo, ewhi, rwT, rb, irep, maskbd


def kernel(x, expert_weight, routing_w, routing_b, trace=False):
    from concourse.bass_utils import run_bass_kernel_spmd

    nc = _get_compiled()
    xlo, xhi, ewlo, ewhi, rwT, rb, irep, maskbd = _host_prep(
        x, expert_weight, routing_w, routing_b
    )
    in_maps = [
        {
            "xlo": np.ascontiguousarray(xlo[i * BS : (i + 1) * BS]),
            "xhi": np.ascontiguousarray(xhi[i * BS : (i + 1) * BS]),
            "ewlo": ewlo,
            "ewhi": ewhi,
            "rwT": rwT,
            "rb": rb,
            "irep": irep,
            "maskbd": maskbd,
        }
        for i in range(N_CORES)
    ]
    res = run_bass_kernel_spmd(
        nc, in_maps, core_ids=list(range(N_CORES)), trace=trace
    )
    out = np.concatenate([res.results[i]["out"] for i in range(N_CORES)], axis=0)
    if trace:
        kernel.last_results = res
    return out


# revision 12
# speedup vs baseline: 1.6563x; 1.0825x over previous
"""CondConv2d Trainium2 kernel (v2).

B=32, C=192, H=W=64, O=192, E=8, 3x3 'same' conv.
Data-parallel over batch: 8 cores x 4 samples. Expert weights replicated.

Key design points vs v1:
  - x is zero-padded to 66x66 on the host (bf16) and stays SBUF-resident:
    one DMA per (sample, channel-half) serves BOTH the avg-pool reduction
    and all conv strip views.  No per-strip DMAs, no pad DMAs.
  - Channels 128..191 are host-packed twice into one 128-partition tile
    (row-shift 0 in partitions 0..63, row-shift +1 in partitions 64..127),
    so taps (kh=0,kh=1) of the hi channel group run as ONE full-contract
    matmul: 15 conv matmuls per strip/o-group instead of 18.
  - expert_weight is host-permuted into mixing order [(g,e) rows,
    (c_sub,taps,o) cols] so the mixing matmul + one partition-split DMA per
    sample directly yields the conv lhsT layout.  No PE transposes.
  - The block-diagonal mixing matrix bd[(g,e),(g,b)] = r[b,e] is built with
    one tiny matmul (identity-tiled lhsT, broadcast rhs) + one masked DVE
    multiply instead of 24 small DMAs.
  - Everything 16-bit on the wire (bf16); PSUM accumulation is f32.
"""

import sys
import numpy as np

for _p in ("/opt/trn_rl_repo",):
    if _p not in sys.path:
        sys.path.insert(0, _p)

import ml_dtypes

BF16 = ml_dtypes.bfloat16

BS = 4          # samples per core
C = 192
H = W = 64
HP = WP = 66    # padded
O = 192
E = 8
KK = 9          # 3x3
N_CORES = 8

G = 16          # mixing channel groups
CSUB = C // G // 1  # 12? no: lo covers 128 ch with CSUB=8 -> see below
CSUB = 8        # channels per group (both passes use 16 groups of 8 partitions)
LO_SPAN = KK * O            # 1728 per-partition cols of lhsT_lo
HI_SPAN = 6 * O             # 1152 per-partition cols of lhsT_hi
LO_COLS = CSUB * LO_SPAN    # 13824 streamed cols, lo mixing pass
HI_COLS = CSUB * HI_SPAN    # 9216 streamed cols, hi mixing pass
MIXN = 512                  # mixing matmul free dim (27 lo tiles, 18 hi tiles)

NS = 8          # row strips per sample
SR = H // NS    # 8 output rows per strip
QROWS = 16      # output rows staged per DMA (2 strips)

# ew stream chunking (units of MIXN columns)
LO_CHUNKS = (7, 7, 7, 6)    # 27 tiles
HI_CHUNKS = (6, 6, 6)       # 18 tiles

_COMPILED = None


def _build():
    import concourse.bass as bass
    import concourse.bacc as bacc
    import concourse.mybir as mybir
    import concourse.tile as tile

    f32 = mybir.dt.float32
    bf16 = mybir.dt.bfloat16
    AX = mybir.AxisListType
    ACT = mybir.ActivationFunctionType
    MUL = mybir.AluOpType.mult

    nc = bacc.Bacc("TRN2", target_bir_lowering=False, debug=False)

    xlo_d = nc.dram_tensor("xlo", [BS, 128, HP, WP], bf16, kind="ExternalInput")
    xhi_d = nc.dram_tensor("xhi", [BS, 128, HP, WP], bf16, kind="ExternalInput")
    ewlo_d = nc.dram_tensor("ewlo", [128, LO_COLS], bf16, kind="ExternalInput")
    ewhi_d = nc.dram_tensor("ewhi", [128, HI_COLS], bf16, kind="ExternalInput")
    rwT_d = nc.dram_tensor("rwT", [C, E], f32, kind="ExternalInput")
    rb_d = nc.dram_tensor("rb", [E], f32, kind="ExternalInput")
    irep_d = nc.dram_tensor("irep", [E, 128], bf16, kind="ExternalInput")
    mask_d = nc.dram_tensor("maskbd", [128, G * BS], bf16, kind="ExternalInput")
    out_d = nc.dram_tensor("out", [BS, O, H, W], f32, kind="ExternalOutput")

    with tile.TileContext(nc) as tc:
        with tc.tile_pool(name="persist", bufs=1) as persist:
            # ---------- constants ----------
            rwT_lo = persist.tile([128, E], f32)
            rwT_hi = persist.tile([64, E], f32)
            nc.sync.dma_start(rwT_lo[:], rwT_d.ap()[0:128])
            nc.sync.dma_start(rwT_hi[:], rwT_d.ap()[128:192])
            rb_t = persist.tile([E, 1], f32)
            nc.sync.dma_start(rb_t[:], rb_d.ap().unsqueeze(1))
            irep = persist.tile([E, 128], bf16)
            nc.sync.dma_start(irep[:], irep_d.ap())
            maskbd = persist.tile([128, G * BS], bf16)
            nc.sync.dma_start(maskbd[:], mask_d.ap())

            # top-row / bottom-row partial pooled sums (DVE / ACT split)
            pooled_t_lo = persist.tile([128, BS], f32)
            pooled_b_lo = persist.tile([128, BS], f32)
            pooled_t_hi = persist.tile([64, BS], f32)
            pooled_b_hi = persist.tile([64, BS], f32)

            # ---------- resident x ----------
            # bank1 of xs_hi (row-shifted duplicate, partitions 64..127) is
            # only needed by the conv, so its loads are deferred below the
            # expert-weight stream.
            xs_lo = [persist.tile([128, HP, WP], bf16, name=f"xs_lo{b}") for b in range(BS)]
            xs_hi = [persist.tile([128, HP, WP], bf16, name=f"xs_hi{b}") for b in range(BS)]
            for b in range(BS):
                nc.sync.dma_start(xs_lo[b][:], xlo_d[b])
                nc.sync.dma_start(xs_hi[b][0:64], xhi_d[b, 0:64])

            # ---------- Phase A reductions (rows 0:32 on DVE, 32:66 on ACT) --
            TOPR = 32
            with (
                tc.tile_pool(name="rpool", bufs=2) as rpool,
            ):
                for b in range(BS):
                    for xt, pt, pb, p_n in (
                        (xs_lo[b], pooled_t_lo, pooled_b_lo, 128),
                        (xs_hi[b], pooled_t_hi, pooled_b_hi, 64),
                    ):
                        sadd = rpool.tile([p_n, TOPR // 2, WP], bf16, tag=f"sadd{p_n}", name="sadd")
                        nc.vector.tensor_add(
                            sadd[:], xt[0:p_n, 0 : TOPR // 2, :], xt[0:p_n, TOPR // 2 : TOPR, :]
                        )
                        nc.vector.reduce_sum(pt[0:p_n, b : b + 1], sadd[:], axis=AX.XY)
                        scp = rpool.tile([p_n, (HP - TOPR) * WP], bf16, tag=f"scp{p_n}", name="scp")
                        nc.scalar.activation(
                            scp[:],
                            xt[0:p_n, TOPR:HP, :].rearrange("p a b -> p (a b)"),
                            ACT.Copy,
                            accum_out=pb[0:p_n, b : b + 1],
                        )

            # ---------- resident expert weights (mixing layout, chunked) ----
            ew_lo_sb = persist.tile([128, LO_COLS], bf16)
            ew_hi_sb = persist.tile([128, HI_COLS], bf16)
            c0 = 0
            for nt in LO_CHUNKS:
                nc.sync.dma_start(
                    ew_lo_sb[:, c0 : c0 + nt * MIXN], ewlo_d.ap()[:, c0 : c0 + nt * MIXN]
                )
                c0 += nt * MIXN
            c0 = 0
            for nt in HI_CHUNKS:
                nc.sync.dma_start(
                    ew_hi_sb[:, c0 : c0 + nt * MIXN], ewhi_d.ap()[:, c0 : c0 + nt * MIXN]
                )
                c0 += nt * MIXN

            # deferred row-shifted xs_hi bank1 loads (needed only by conv)
            for b in range(BS):
                nc.sync.dma_start(xs_hi[b][64:128], xhi_d[b, 64:128])

            # per-sample conv weights, already transposed layout
            lhsT_lo = [persist.tile([128, LO_SPAN], bf16, name=f"lhsT_lo{b}") for b in range(BS)]
            lhsT_hi = [persist.tile([128, HI_SPAN], bf16, name=f"lhsT_hi{b}") for b in range(BS)]

            with (
                tc.tile_pool(name="spool", bufs=1) as spool,
                tc.tile_pool(name="mixpsum", bufs=4, space="PSUM") as mixpsum,
                tc.tile_pool(name="smallpsum", bufs=1, space="PSUM") as smallpsum,
            ):
                # ---------- routing ----------
                psum_r = smallpsum.tile([E, BS], f32, tag="psum_r")
                nc.tensor.matmul(psum_r[:], rwT_lo[:], pooled_t_lo[:], start=True, stop=False)
                nc.tensor.matmul(psum_r[:], rwT_lo[:], pooled_b_lo[:], start=False, stop=False)
                nc.tensor.matmul(psum_r[:], rwT_hi[:], pooled_t_hi[:], start=False, stop=False)
                nc.tensor.matmul(psum_r[:], rwT_hi[:], pooled_b_hi[:], start=False, stop=True)
                r_sb = persist.tile([E, BS], bf16)
                nc.scalar.activation(
                    r_sb[:], psum_r[:], ACT.Sigmoid, bias=rb_t[:], scale=1.0 / (H * W)
                )

                # ---------- block-diagonal bd[(g,e), (g,b)] = r[b, e] ----------
                psum_bd = smallpsum.tile([128, G * BS], f32, tag="psum_bd")
                r_bcast = r_sb[:].unsqueeze(1).broadcast_to([E, G, BS])
                nc.tensor.matmul(psum_bd[:], irep[:], r_bcast, start=True, stop=True)
                bd = persist.tile([128, G * BS], bf16)
                nc.vector.tensor_tensor(bd[:], psum_bd[:], maskbd[:], MUL)

                # ---------- mixing ----------
                S_lo = spool.tile([G * BS, LO_COLS], bf16, name="S_lo")
                S_hi = spool.tile([G * BS, HI_COLS], bf16, name="S_hi")
                for t in range(LO_COLS // MIXN):
                    pm = mixpsum.tile([G * BS, MIXN], f32, tag="pm")
                    nc.tensor.matmul(
                        pm[:], bd[:], ew_lo_sb[:, t * MIXN : (t + 1) * MIXN],
                        start=True, stop=True,
                    )
                    dst = S_lo[:, t * MIXN : (t + 1) * MIXN]
                    if t % 2 == 0:
                        nc.vector.tensor_copy(dst, pm[:])
                    else:
                        nc.scalar.copy(dst, pm[:])
                # remap rows (g,b) -> lhsT_lo[b] partitions (g, c_sub)
                for b in range(BS):
                    src = S_lo[:].rearrange(
                        "(g b) (c u) -> g b c u", b=BS, c=CSUB
                    )[:, b]
                    nc.sync.dma_start(lhsT_lo[b][:], src)

                for t in range(HI_COLS // MIXN):
                    pm = mixpsum.tile([G * BS, MIXN], f32, tag="pm")
                    nc.tensor.matmul(
                        pm[:], bd[:], ew_hi_sb[:, t * MIXN : (t + 1) * MIXN],
                        start=True, stop=True,
                    )
                    dst = S_hi[:, t * MIXN : (t + 1) * MIXN]
                    if t % 2 == 0:
                        nc.vector.tensor_copy(dst, pm[:])
                    else:
                        nc.scalar.copy(dst, pm[:])
                for b in range(BS):
                    src = S_hi[:].rearrange(
                        "(g b) (c u) -> g b c u", b=BS, c=CSUB
                    )[:, b]
                    nc.sync.dma_start(lhsT_hi[b][:], src)

            # ---------- conv ----------
            with (
                tc.tile_pool(name="stgpool", bufs=2) as stgpool,
                tc.tile_pool(name="cpsum", bufs=3, space="PSUM") as cpsum,
            ):
                SPQ = QROWS // SR            # strips per staged group
                for s in range(BS):
                    for q in range(NS // SPQ):   # staged output groups
                        stgs = {}
                        for oc, (o0, o_n) in enumerate(((0, 128), (128, 64))):
                            stgs[oc] = stgpool.tile(
                                [o_n, QROWS, W], f32, tag=f"stg{oc}", name=f"stg{oc}"
                            )
                        for ui in range(SPQ):
                            u = q * SPQ + ui
                            h0 = u * SR
                            for oc, (o0, o_n) in enumerate(((0, 128), (128, 64))):
                                pc = cpsum.tile([o_n, SR, W], f32, tag=f"pc{oc}")
                                i = 0
                                n_acc = 15
                                for kh in range(3):
                                    for kw in range(3):
                                        kk = 3 * kh + kw
                                        nc.tensor.matmul(
                                            pc[:],
                                            lhsT_lo[s][:, kk * O + o0 : kk * O + o0 + o_n],
                                            xs_lo[s][:, h0 + kh : h0 + kh + SR, kw : kw + W],
                                            start=(i == 0), stop=(i == n_acc - 1),
                                        )
                                        i += 1
                                for a_idx, a in enumerate((0, 2)):
                                    for kw in range(3):
                                        j = 3 * a_idx + kw
                                        nc.tensor.matmul(
                                            pc[:],
                                            lhsT_hi[s][:, j * O + o0 : j * O + o0 + o_n],
                                            xs_hi[s][:, h0 + a : h0 + a + SR, kw : kw + W],
                                            start=(i == 0), stop=(i == n_acc - 1),
                                        )
                                        i += 1
                                nc.vector.tensor_copy(
                                    stgs[oc][:, ui * SR : (ui + 1) * SR, :], pc[:]
                                )
                        for oc, (o0, o_n) in enumerate(((0, 128), (128, 64))):
                            nc.sync.dma_start(
                                out_d[s, o0 : o0 + o_n, q * QROWS : (q + 1) * QROWS, :],
                                stgs[oc][:],
                            )

    nc.compile()
    return nc


def _get_compiled():
    global _COMPILED
    if _COMPILED is None:
        _COMPILED = _build()
    return _COMPILED


def _host_prep(x, expert_weight, routing_w, routing_b):
    """Pure layout transforms (pad/transpose/cast); no model arithmetic."""
    B = x.shape[0]
    x = np.asarray(x, dtype=np.float32)

    # padded lo channels [B, 128, 66, 66]
    xlo = np.zeros((B, 128, HP, WP), dtype=np.float32)
    xlo[:, :, 1 : H + 1, 1 : W + 1] = x[:, 0:128]
    # hi channels padded, plus row-shifted copy stacked on partitions 64..127
    xhi_p = np.zeros((B, 64, HP, WP), dtype=np.float32)
    xhi_p[:, :, 1 : H + 1, 1 : W + 1] = x[:, 128:192]
    xhi_r = np.zeros_like(xhi_p)
    xhi_r[:, :, 0 : HP - 1, :] = xhi_p[:, :, 1:HP, :]
    xhi = np.concatenate([xhi_p, xhi_r], axis=1)

    xlo = xlo.astype(BF16)
    xhi = xhi.astype(BF16)

    # expert weights -> [E, C, KH, KW, O]
    Wt = np.ascontiguousarray(
        np.asarray(expert_weight, dtype=np.float32)
        .reshape(E, O, C, 3, 3)
        .transpose(0, 2, 3, 4, 1)
    )
    # lo: channels 0..127, per-channel (kh, kw, o); mixing-partition-major:
    # SBUF partition (g*8+e) holds expert e's channels g*8..g*8+7
    ewlo = (
        np.ascontiguousarray(Wt[:, 0:128])
        .reshape(E, G, CSUB * LO_SPAN)
        .transpose(1, 0, 2)
        .reshape(128, LO_COLS)
        .astype(BF16)
    )
    # hi: bank0 = (kh0 taps, kh2 taps) for c128..191; bank1 = (kh1 taps, zeros)
    hi = Wt[:, 128:192]                       # [E, 64, 3, 3, O]
    bank0 = np.concatenate([hi[:, :, 0], hi[:, :, 2]], axis=2)   # [E, 64, 6, O]
    bank1 = np.concatenate(
        [hi[:, :, 1], np.zeros((E, 64, 3, O), dtype=np.float32)], axis=2
    )
    ewhi = (
        np.concatenate([bank0, bank1], axis=1)  # [E, 128, 6, O]
        .reshape(E, G, CSUB * HI_SPAN)
        .transpose(1, 0, 2)
        .reshape(128, HI_COLS)
        .astype(BF16)
    )

    rwT = np.ascontiguousarray(np.asarray(routing_w, dtype=np.float32).T)
    rb = np.ascontiguousarray(np.asarray(routing_b, dtype=np.float32))

    irep = np.ascontiguousarray(np.tile(np.eye(E, dtype=np.float32), (1, G))).astype(BF16)
    maskbd = np.zeros((128, G * BS), dtype=np.float32)
    for g in range(G):
        maskbd[g * E : (g + 1) * E, g * BS : (g + 1) * BS] = 1.0
    maskbd = maskbd.astype(BF16)

    return xlo, xhi, ewlo, ewhi, rwT, rb, irep, maskbd


def kernel(x, expert_weight, routing_w, routing_b, trace=False):
    from concourse.bass_utils import run_bass_kernel_spmd

    nc = _get_compiled()
    xlo, xhi, ewlo, ewhi, rwT, rb, irep, maskbd = _host_prep(
        x, expert_weight, routing_w, routing_b
    )
    in_maps = [
        {
            "xlo": np.ascontiguousarray(xlo[i * BS : (i + 1) * BS]),
            "xhi": np.ascontiguousarray(xhi[i * BS : (i + 1) * BS]),
            "ewlo": ewlo,
            "ewhi": ewhi,
            "rwT": rwT,
            "rb": rb,
            "irep": irep,
            "maskbd": maskbd,
        }
        for i in range(N_CORES)
    ]
    res = run_bass_kernel_spmd(
        nc, in_maps, core_ids=list(range(N_CORES)), trace=trace
    )
    out = np.concatenate([res.results[i]["out"] for i in range(N_CORES)], axis=0)
    if trace:
        kernel.last_results = res
    return out
